# revision 45
# baseline (speedup 1.0000x reference)
"""Trainium2 Bass kernel for segment_reduce (Raw2Alpha + Alphas2Weights).

Math (per sample i of ray r, interval = 1):
    explp  = 1 - alpha = sigmoid(-(density + shift))      # = exp(log(1-alpha))
    alpha  = sigmoid(density + shift)
    T      = exclusive within-ray cumprod of explp        # transmittance
    weights          = T * alpha
    alphainv_last[r] = prod of explp over ray r

Device strategy (pure data parallel over rays, 8 cores):
  * Host packs rays into fixed-width slots of L columns in a [128, F]
    layout (F = spr*L, spr slots per partition row).  Rays longer than
    L-1 samples are split across ADJACENT slots in the same row/chunk.
    Pads use density=-30 so explp == 1.0 exactly (multiplicative identity).
  * One DVE tensor_tensor_scan per tile computes the exclusive segmented
    cumprod:  state = data0*state + data1, with
       data0 = explp shifted right one column; at slot-start columns it
               holds the host "continuation mask" (0.0 = reset = new ray,
               1.0 = ray continues from previous slot);
       data1 = (1 - mask) at slot starts (re-seeds state to 1), 0 elsewhere.
    For continuation slots the running product simply flows through.
  * alpha via a second sigmoid (same ACT table set — no table thrash).
  * Per-ray totals land at slot-end columns: strided slice, no gather.
"""

import math
import os
from collections import deque

import numpy as np

import concourse.bacc as bacc
import concourse.mybir as mybir
import concourse.tile as tile
from concourse.bass_utils import run_bass_kernel_spmd

F32 = mybir.dt.float32
N_CORES = 8
P = 128  # partitions
PAD_VAL = -30.0  # sigmoid(-(PAD_VAL+shift)) rounds to exactly 1.0f for |shift|<10

_nc_cache = {}


def _chunk_plan(spr, first, spc):
    """Slot ranges: a small first chunk to fill the pipeline fast, then
    spc-slot chunks."""
    bounds = [0, min(first, spr)]
    while bounds[-1] < spr:
        bounds.append(min(bounds[-1] + spc, spr))
    # keep the LAST chunk small too: its scan->mult->store chain is the
    # kernel's drain tail
    if len(bounds) >= 3 and bounds[-1] - bounds[-2] > first:
        bounds.insert(-1, bounds[-1] - first)
    return list(zip(bounds[:-1], bounds[1:]))


def _build_nc(L, spr, first, spc, shift, interval):
    """Bass program: L slot width, spr slots/row, chunk plan (first, spc)."""
    F = spr * L
    plan = _chunk_plan(spr, first, spc)

    nc = bacc.Bacc(
        "TRN2", target_bir_lowering=False, debug=False, num_devices=N_CORES
    )
    # Drop the const-AP preamble memsets (4 slow GPSIMD ops, ~4us): no
    # instruction in this kernel reads the const-AP database (all activation
    # biases below are explicit APs or Copy-immediates).
    _blk = nc.m.functions[0].blocks[0]

    def _strip(i):
        tn = type(i).__name__
        if tn == "InstMemset":
            return True
        # the ~3.4us entry all-engine barrier only ordered those memsets
        if tn in ("InstDrain", "InstEventSemaphore"):
            return True
        # GPSIMD/PE execute nothing in this kernel except semaphore ops
        if tn in ("InstRegisterMove", "InstTPBBaseLd") and str(i.engine) in (
            "EngineType.Pool",
            "EngineType.PE",
        ):
            return True
        return False

    _blk.instructions = [i for i in _blk.instructions if not _strip(i)]
    den = nc.dram_tensor("density", [P, F], F32, kind="ExternalInput")
    maskin = nc.dram_tensor("contmask", [P, spr], F32, kind="ExternalInput")
    wout = nc.dram_tensor("weights", [P, F], F32, kind="ExternalOutput")
    aiout = nc.dram_tensor("alphainv", [P, spr], F32, kind="ExternalOutput")

    sig = mybir.ActivationFunctionType.Sigmoid
    copy_fn = mybir.ActivationFunctionType.Copy
    use_sigmoid = interval == 1.0
    CHM = max(s1 - s0 for s0, s1 in plan) * L

    with tile.TileContext(nc) as tc:
        with (
            tc.tile_pool(name="io", bufs=4) as io,
            tc.tile_pool(name="work", bufs=3) as work,
            tc.tile_pool(name="const", bufs=1) as const,
        ):
            maskt = const.tile([P, spr], F32)
            nc.sync.dma_start(maskt[:], maskin[:])
            onem = const.tile([P, spr], F32)  # 1 - mask
            nc.vector.tensor_scalar(
                onem[:], maskt[:], -1.0, 1.0, mybir.AluOpType.mult, mybir.AluOpType.add
            )
            d1c = const.tile([P, CHM], F32)  # scan data1: zeros + seeds at starts
            nc.vector.memset(d1c[:], 0.0)
            ai = const.tile([P, spr], F32)
            # Bias constants built on the ACT engine (same-engine deps are
            # free; Copy takes immediate bias, needing no const-AP database).
            bpos = const.tile([P, 1], F32)
            nc.scalar.activation(bpos[:], bpos[:], copy_fn, bias=shift, scale=0.0)
            bneg = const.tile([P, 1], F32)
            nc.scalar.activation(bneg[:], bpos[:], copy_fn, bias=-shift, scale=0.0)
            bzero = const.tile([P, 1], F32)
            nc.scalar.activation(bzero[:], bpos[:], copy_fn, bias=0.0, scale=0.0)
            # Warm up the sigmoid table set with a dependency-free op so the
            # ACT_TABLE_LOAD wait doesn't ride on a real activation.
            warm = const.tile([P, 1], F32)
            nc.scalar.activation(warm[:], warm[:], sig, bias=bneg[:], scale=1.0)

            pend_ai = []
            for ci, (s0, s1) in enumerate(plan):
                ns = s1 - s0
                w = ns * L
                sl = slice(s0 * L, s1 * L)
                d = io.tile([P, CHM], F32, tag="d")
                # first chunk: issue from ACT's HWDGE queue — ACT's sequencer
                # is ready ~2.5us before SP, shortening the pipeline fill
                dma_eng = nc.scalar if ci == 0 else nc.sync
                dma_eng.dma_start(d[:, :w], den[:, sl])

                # ez[:, j] holds explp of sample j-1 (shifted)
                ez = work.tile([P, CHM + 1], F32, tag="ez")
                if use_sigmoid:
                    nc.scalar.activation(
                        ez[:, 1 : w + 1], d[:, :w], sig, bias=bneg[:], scale=-1.0
                    )
                else:
                    # explp = sigmoid(-(x+shift))**interval via ln/exp
                    # (no softplus table in this environment; ln+exp share
                    # one table set)
                    sp = work.tile([P, CHM], F32, tag="sp")
                    nc.scalar.activation(
                        sp[:, :w], d[:, :w], sig, bias=bneg[:], scale=-1.0
                    )
                    nc.scalar.activation(
                        sp[:, :w],
                        sp[:, :w],
                        mybir.ActivationFunctionType.Ln,
                        bias=bzero[:],
                        scale=1.0,
                    )
                    nc.scalar.activation(
                        ez[:, 1 : w + 1],
                        sp[:, :w],
                        mybir.ActivationFunctionType.Exp,
                        bias=bzero[:],
                        scale=interval,
                    )
                # slot-start columns: 0.0 resets a new ray, 1.0 continues one
                nc.vector.tensor_copy(ez[:, 0:w:L], maskt[:, s0:s1])
                # scan data1: 1-mask seeds state=1 at resets
                nc.vector.tensor_copy(d1c[:, 0:w:L], onem[:, s0:s1])

                # alpha at full precision
                al = work.tile([P, CHM], F32, tag="al")
                if use_sigmoid:
                    nc.scalar.activation(
                        al[:, :w], d[:, :w], sig, bias=bpos[:], scale=1.0
                    )
                else:
                    nc.vector.tensor_scalar(
                        al[:, :w],
                        ez[:, 1 : w + 1],
                        -1.0,
                        1.0,
                        mybir.AluOpType.mult,
                        mybir.AluOpType.add,
                    )

                # exclusive segmented cumprod: state = ez*state + d1c
                t = work.tile([P, CHM], F32, tag="t")
                nc.vector.tensor_tensor_scan(
                    t[:, :w],
                    ez[:, 0:w],
                    d1c[:, 0:w],
                    0.0,
                    mybir.AluOpType.mult,
                    mybir.AluOpType.add,
                )

                # weights = alpha * T, 3/4 on DVE + 1/4 on GPSIMD (short
                # GPSIMD bursts bound the SBUF-port contention with the scan)
                wt = io.tile([P, CHM], F32, tag="w")
                h = (w * 3 // 4) // 4 * 4
                nc.vector.tensor_tensor(
                    wt[:, :h], al[:, :h], t[:, :h], mybir.AluOpType.mult
                )
                nc.gpsimd.tensor_tensor(
                    wt[:, h:w], al[:, h:w], t[:, h:w], mybir.AluOpType.mult
                )
                nc.sync.dma_start(wout[:, sl], wt[:, :w])

                # per-ray running totals sit at slot-end columns.  Extract on
                # ACT, but DEFERRED one chunk: an in-order ACT copy would wait
                # on this chunk's DVE scan from inside ACT's stream, stalling
                # the next chunk's sigmoid.
                pend_ai.append((s0, s1, w, t))
                if len(pend_ai) > 1:
                    ps0, ps1, pw, pt = pend_ai.pop(0)
                    nc.scalar.copy(ai[:, ps0:ps1], pt[:, L - 1 : pw : L])

            for ps0, ps1, pw, pt in pend_ai:
                nc.scalar.copy(ai[:, ps0:ps1], pt[:, L - 1 : pw : L])
            nc.sync.dma_start(aiout[:], ai[:])
    nc.compile()
    return nc


def _pack_core(nch, spr, starts_disallowed):
    """Assign each ray's chunks to consecutive slots of a [128, spr] grid.

    nch: per-ray chunk counts (1 or 2).  Multi-chunk rays must occupy
    adjacent slots within one row, not straddling a chunk boundary
    (starts_disallowed: slot indices that may not be a continuation).
    Returns (row[r], slot0[r]) or None if it doesn't fit in 128 rows.
    """
    n = len(nch)
    row = np.empty(n, dtype=np.int32)
    slot0 = np.empty(n, dtype=np.int32)
    pend = deque(range(n))
    r_i, s_i = 0, 0
    while pend:
        if r_i >= P:
            return None
        ray = pend[0]
        c = nch[ray]
        ok = s_i + c <= spr and (c == 1 or (s_i + 1) not in starts_disallowed)
        if ok:
            pend.popleft()
            row[ray] = r_i
            slot0[ray] = s_i
            s_i += c
        else:
            # find a later single-chunk ray to fill this slot
            filled = False
            for k in range(1, min(len(pend), 64)):
                alt = pend[k]
                if nch[alt] == 1:
                    del pend[k]
                    row[alt] = r_i
                    slot0[alt] = s_i
                    s_i += 1
                    filled = True
                    break
            if not filled:
                s_i += 1  # waste the slot
        if s_i >= spr:
            r_i += 1
            s_i = 0
    return row, slot0


def kernel(density, shift, interval, ray_id, n_rays, _bench=None):
    density = np.asarray(density, dtype=np.float32).reshape(-1)
    ray_id = np.asarray(ray_id).astype(np.int64).reshape(-1)
    shift_f = float(np.asarray(shift))
    interval_f = float(np.asarray(interval))
    n_rays = int(n_rays)
    M = density.shape[0]

    # ---- host-side layout (derived only from ray_id) ----
    rays_per_core = math.ceil(n_rays / N_CORES)
    rays_padded = N_CORES * rays_per_core

    counts = np.bincount(ray_id, minlength=rays_padded).astype(np.int64)
    maxlen = max(int(counts.max()), 1) if M else 1
    first, spc = 4, 10

    # choose slot width L minimizing F = spr*L (pads vs splits trade-off)
    best = None
    for L in range(((maxlen + 4) // 4) * 4 + 4, 4, -4):
        cap = L - 1
        if maxlen > 2 * cap:
            break  # keep at most 2 chunks per ray
        worst = 0
        for c in range(N_CORES):
            cc = counts[c * rays_per_core : (c + 1) * rays_per_core]
            slots = int(np.maximum(1, -(-cc // cap)).sum())
            worst = max(worst, -(-slots // P))
        F = worst * L
        if best is None or F < best[0]:
            best = (F, L, worst)
    _, L, spr = best
    cap = L - 1

    # pack every core; grow spr on (rare) packing overflow
    packs = []
    while True:
        bounds = {s0 for s0, _ in _chunk_plan(spr, first, spc)}
        ok = True
        packs = []
        for c in range(N_CORES):
            cc = counts[c * rays_per_core : (c + 1) * rays_per_core]
            nch = np.maximum(1, -(-cc // cap)).astype(np.int32)
            got = _pack_core(nch, spr, bounds)
            if got is None:
                ok = False
                break
            packs.append(got)
        if ok:
            break
        spr += 1
    F = spr * L

    key = (L, spr, first, spc, shift_f, interval_f)
    if key not in _nc_cache:
        _nc_cache[key] = _build_nc(L, spr, first, spc, shift_f, interval_f)
    nc = _nc_cache[key]

    # per-ray placement arrays (global)
    g_row = np.empty(rays_padded, dtype=np.int64)
    g_slot0 = np.empty(rays_padded, dtype=np.int64)
    for c in range(N_CORES):
        row, slot0 = packs[c]
        g_row[c * rays_per_core : (c + 1) * rays_per_core] = row + c * P
        g_slot0[c * rays_per_core : (c + 1) * rays_per_core] = slot0

    # per-sample positions in the concatenated [8*128, F] padded array
    starts = np.zeros(rays_padded + 1, dtype=np.int64)
    np.cumsum(counts, out=starts[1:])
    off = np.arange(M, dtype=np.int64) - starts[ray_id]
    pos = (
        g_row[ray_id] * F
        + (g_slot0[ray_id] + off // cap) * L
        + off % cap
    )

    padded = np.full(N_CORES * P * F, PAD_VAL, dtype=np.float32)
    padded[pos] = density
    padded = padded.reshape(N_CORES, P, F)

    # continuation mask: 1.0 at every non-first chunk slot of a ray
    contmask = np.zeros((N_CORES * P, spr), dtype=np.float32)
    multi = np.nonzero(counts > cap)[0]
    for r in multi:
        for j in range(1, int(-(-counts[r] // cap))):
            contmask[g_row[r], g_slot0[r] + j] = 1.0
    contmask = contmask.reshape(N_CORES, P, spr)

    in_maps = [
        {"density": padded[c], "contmask": contmask[c]} for c in range(N_CORES)
    ]
    res = run_bass_kernel_spmd(
        nc,
        in_maps,
        core_ids=list(range(N_CORES)),
        trace=bool(os.environ.get("BASS_TRACE")),
    )
    if _bench is not None:
        _bench.append(res)

    w_all = np.stack([res.results[c]["weights"] for c in range(N_CORES)])
    weights = np.ascontiguousarray(w_all.reshape(-1)[pos])
    # alphainv: running product at the ray's LAST chunk's slot end
    ai_all = np.stack([res.results[c]["alphainv"] for c in range(N_CORES)])
    nch_all = np.maximum(1, -(-counts // cap)).astype(np.int64)
    ai_pos = g_row * spr + g_slot0 + nch_all - 1
    alphainv = np.ascontiguousarray(ai_all.reshape(-1)[ai_pos][:n_rays])
    return weights, alphainv


# revision 46
# speedup vs baseline: 1.1197x; 1.1197x over previous
"""Trainium2 Bass kernel for segment_reduce (Raw2Alpha + Alphas2Weights).

Math (per sample i of ray r, interval = 1):
    explp  = 1 - alpha = sigmoid(-(density + shift))      # = exp(log(1-alpha))
    alpha  = sigmoid(density + shift)
    T      = exclusive within-ray cumprod of explp        # transmittance
    weights          = T * alpha
    alphainv_last[r] = prod of explp over ray r

Device strategy (pure data parallel over rays, 8 cores):
  * Host packs rays into fixed-width slots of L columns in a [128, F]
    layout (F = spr*L, spr slots per partition row).  Rays longer than
    L-1 samples are split across ADJACENT slots in the same row/chunk.
    Pads use density=-30 so explp == 1.0 exactly (multiplicative identity).
  * One DVE tensor_tensor_scan per tile computes the exclusive segmented
    cumprod:  state = data0*state + data1, with
       data0 = explp shifted right one column; at slot-start columns it
               holds the host "continuation mask" (0.0 = reset = new ray,
               1.0 = ray continues from previous slot);
       data1 = (1 - mask) at slot starts (re-seeds state to 1), 0 elsewhere.
    For continuation slots the running product simply flows through.
  * alpha via a second sigmoid (same ACT table set — no table thrash).
  * Per-ray totals land at slot-end columns: strided slice, no gather.
"""

import math
import os
from collections import deque

import numpy as np

import concourse.bacc as bacc
import concourse.mybir as mybir
import concourse.tile as tile
from concourse.bass_utils import run_bass_kernel_spmd

F32 = mybir.dt.float32
N_CORES = 8
P = 128  # partitions
PAD_VAL = -30.0  # sigmoid(-(PAD_VAL+shift)) rounds to exactly 1.0f for |shift|<10

_nc_cache = {}


def _chunk_plan(spr, first, spc):
    """Slot ranges: a small first chunk to fill the pipeline fast, then
    spc-slot chunks."""
    bounds = [0, min(first, spr)]
    while bounds[-1] < spr:
        bounds.append(min(bounds[-1] + spc, spr))
    # keep the LAST chunk small too: its scan->mult->store chain is the
    # kernel's drain tail
    if len(bounds) >= 3 and bounds[-1] - bounds[-2] > first:
        bounds.insert(-1, bounds[-1] - first)
    return list(zip(bounds[:-1], bounds[1:]))


def _build_nc(L, spr, first, spc, shift, interval):
    """Bass program: L slot width, spr slots/row, chunk plan (first, spc)."""
    F = spr * L
    plan = _chunk_plan(spr, first, spc)

    nc = bacc.Bacc(
        "TRN2", target_bir_lowering=False, debug=False, num_devices=N_CORES
    )
    # Drop the const-AP preamble memsets (4 slow GPSIMD ops, ~4us): no
    # instruction in this kernel reads the const-AP database (all activation
    # biases below are explicit APs or Copy-immediates).
    _blk = nc.m.functions[0].blocks[0]

    def _strip(i):
        tn = type(i).__name__
        if tn == "InstMemset":
            return True
        # the ~3.4us entry all-engine barrier only ordered those memsets
        if tn in ("InstDrain", "InstEventSemaphore"):
            return True
        # GPSIMD/PE execute nothing in this kernel except semaphore ops
        if tn in ("InstRegisterMove", "InstTPBBaseLd") and str(i.engine) in (
            "EngineType.Pool",
            "EngineType.PE",
        ):
            return True
        return False

    _blk.instructions = [i for i in _blk.instructions if not _strip(i)]
    den = nc.dram_tensor("density", [P, F], F32, kind="ExternalInput")
    maskin = nc.dram_tensor("contmask", [P, spr], F32, kind="ExternalInput")
    wout = nc.dram_tensor("weights", [P, F], F32, kind="ExternalOutput")
    aiout = nc.dram_tensor("alphainv", [P, spr], F32, kind="ExternalOutput")

    sig = mybir.ActivationFunctionType.Sigmoid
    copy_fn = mybir.ActivationFunctionType.Copy
    use_sigmoid = interval == 1.0
    CHM = max(s1 - s0 for s0, s1 in plan) * L

    with tile.TileContext(nc) as tc:
        with (
            tc.tile_pool(name="io", bufs=4) as io,
            tc.tile_pool(name="work", bufs=3) as work,
            tc.tile_pool(name="const", bufs=1) as const,
        ):
            maskt = const.tile([P, spr], F32)
            nc.sync.dma_start(maskt[:], maskin[:])
            onem = const.tile([P, spr], F32)  # 1 - mask
            nc.vector.tensor_scalar(
                onem[:], maskt[:], -1.0, 1.0, mybir.AluOpType.mult, mybir.AluOpType.add
            )
            d1c = const.tile([P, CHM], F32)  # scan data1: zeros + seeds at starts
            nc.vector.memset(d1c[:], 0.0)
            ai = const.tile([P, spr], F32)
            # Bias constants built on the ACT engine (same-engine deps are
            # free; Copy takes immediate bias, needing no const-AP database).
            bpos = const.tile([P, 1], F32)
            nc.scalar.activation(bpos[:], bpos[:], copy_fn, bias=shift, scale=0.0)
            bneg = const.tile([P, 1], F32)
            nc.scalar.activation(bneg[:], bpos[:], copy_fn, bias=-shift, scale=0.0)
            bzero = const.tile([P, 1], F32)
            nc.scalar.activation(bzero[:], bpos[:], copy_fn, bias=0.0, scale=0.0)
            # Warm up the sigmoid table set with a dependency-free op so the
            # ACT_TABLE_LOAD wait doesn't ride on a real activation.
            warm = const.tile([P, 1], F32)
            nc.scalar.activation(warm[:], warm[:], sig, bias=bneg[:], scale=1.0)

            pend_ai = []
            for ci, (s0, s1) in enumerate(plan):
                ns = s1 - s0
                w = ns * L
                sl = slice(s0 * L, s1 * L)
                d = io.tile([P, CHM], F32, tag="d")
                # first chunk: issue from ACT's HWDGE queue — ACT's sequencer
                # is ready ~2.5us before SP, shortening the pipeline fill
                dma_eng = nc.scalar if ci == 0 else nc.sync
                dma_eng.dma_start(d[:, :w], den[:, sl])

                # ez[:, j] holds explp of sample j-1 (shifted)
                ez = work.tile([P, CHM + 1], F32, tag="ez")
                if use_sigmoid:
                    nc.scalar.activation(
                        ez[:, 1 : w + 1], d[:, :w], sig, bias=bneg[:], scale=-1.0
                    )
                else:
                    # explp = sigmoid(-(x+shift))**interval via ln/exp
                    # (no softplus table in this environment; ln+exp share
                    # one table set)
                    sp = work.tile([P, CHM], F32, tag="sp")
                    nc.scalar.activation(
                        sp[:, :w], d[:, :w], sig, bias=bneg[:], scale=-1.0
                    )
                    nc.scalar.activation(
                        sp[:, :w],
                        sp[:, :w],
                        mybir.ActivationFunctionType.Ln,
                        bias=bzero[:],
                        scale=1.0,
                    )
                    nc.scalar.activation(
                        ez[:, 1 : w + 1],
                        sp[:, :w],
                        mybir.ActivationFunctionType.Exp,
                        bias=bzero[:],
                        scale=interval,
                    )
                # slot-start columns: 0.0 resets a new ray, 1.0 continues one
                nc.vector.tensor_copy(ez[:, 0:w:L], maskt[:, s0:s1])
                # scan data1: 1-mask seeds state=1 at resets
                nc.vector.tensor_copy(d1c[:, 0:w:L], onem[:, s0:s1])

                # alpha at full precision
                al = work.tile([P, CHM], F32, tag="al")
                if use_sigmoid:
                    nc.scalar.activation(
                        al[:, :w], d[:, :w], sig, bias=bpos[:], scale=1.0
                    )
                else:
                    nc.vector.tensor_scalar(
                        al[:, :w],
                        ez[:, 1 : w + 1],
                        -1.0,
                        1.0,
                        mybir.AluOpType.mult,
                        mybir.AluOpType.add,
                    )

                # exclusive segmented cumprod: state = ez*state + d1c
                t = work.tile([P, CHM], F32, tag="t")
                nc.vector.tensor_tensor_scan(
                    t[:, :w],
                    ez[:, 0:w],
                    d1c[:, 0:w],
                    0.0,
                    mybir.AluOpType.mult,
                    mybir.AluOpType.add,
                )

                # weights = alpha * T (all-DVE: any concurrent GPSIMD
                # streaming slows the DVE scan via SBUF-port contention)
                wt = io.tile([P, CHM], F32, tag="w")
                nc.vector.tensor_tensor(
                    wt[:, :w], al[:, :w], t[:, :w], mybir.AluOpType.mult
                )
                nc.sync.dma_start(wout[:, sl], wt[:, :w])

                # per-ray running totals sit at slot-end columns.  Extract on
                # ACT, but DEFERRED one chunk: an in-order ACT copy would wait
                # on this chunk's DVE scan from inside ACT's stream, stalling
                # the next chunk's sigmoid.
                pend_ai.append((s0, s1, w, t))
                if len(pend_ai) > 1:
                    ps0, ps1, pw, pt = pend_ai.pop(0)
                    nc.scalar.copy(ai[:, ps0:ps1], pt[:, L - 1 : pw : L])

            for ps0, ps1, pw, pt in pend_ai:
                nc.scalar.copy(ai[:, ps0:ps1], pt[:, L - 1 : pw : L])
            nc.sync.dma_start(aiout[:], ai[:])
    nc.compile()
    return nc


def _pack_core(nch, spr, starts_disallowed):
    """Assign each ray's chunks to consecutive slots of a [128, spr] grid.

    nch: per-ray chunk counts (1 or 2).  Multi-chunk rays must occupy
    adjacent slots within one row, not straddling a chunk boundary
    (starts_disallowed: slot indices that may not be a continuation).
    Returns (row[r], slot0[r]) or None if it doesn't fit in 128 rows.
    """
    n = len(nch)
    row = np.empty(n, dtype=np.int32)
    slot0 = np.empty(n, dtype=np.int32)
    pend = deque(range(n))
    r_i, s_i = 0, 0
    while pend:
        if r_i >= P:
            return None
        ray = pend[0]
        c = nch[ray]
        ok = s_i + c <= spr and (c == 1 or (s_i + 1) not in starts_disallowed)
        if ok:
            pend.popleft()
            row[ray] = r_i
            slot0[ray] = s_i
            s_i += c
        else:
            # find a later single-chunk ray to fill this slot
            filled = False
            for k in range(1, min(len(pend), 64)):
                alt = pend[k]
                if nch[alt] == 1:
                    del pend[k]
                    row[alt] = r_i
                    slot0[alt] = s_i
                    s_i += 1
                    filled = True
                    break
            if not filled:
                s_i += 1  # waste the slot
        if s_i >= spr:
            r_i += 1
            s_i = 0
    return row, slot0


def kernel(density, shift, interval, ray_id, n_rays, _bench=None):
    density = np.asarray(density, dtype=np.float32).reshape(-1)
    ray_id = np.asarray(ray_id).astype(np.int64).reshape(-1)
    shift_f = float(np.asarray(shift))
    interval_f = float(np.asarray(interval))
    n_rays = int(n_rays)
    M = density.shape[0]

    # ---- host-side layout (derived only from ray_id) ----
    rays_per_core = math.ceil(n_rays / N_CORES)
    rays_padded = N_CORES * rays_per_core

    counts = np.bincount(ray_id, minlength=rays_padded).astype(np.int64)
    maxlen = max(int(counts.max()), 1) if M else 1
    first, spc = 4, 10

    # choose slot width L minimizing F = spr*L (pads vs splits trade-off)
    best = None
    for L in range(((maxlen + 4) // 4) * 4 + 4, 4, -4):
        cap = L - 1
        if maxlen > 2 * cap:
            break  # keep at most 2 chunks per ray
        worst = 0
        for c in range(N_CORES):
            cc = counts[c * rays_per_core : (c + 1) * rays_per_core]
            slots = int(np.maximum(1, -(-cc // cap)).sum())
            worst = max(worst, -(-slots // P))
        F = worst * L
        if best is None or F < best[0]:
            best = (F, L, worst)
    _, L, spr = best
    cap = L - 1

    # pack every core; grow spr on (rare) packing overflow
    packs = []
    while True:
        bounds = {s0 for s0, _ in _chunk_plan(spr, first, spc)}
        ok = True
        packs = []
        for c in range(N_CORES):
            cc = counts[c * rays_per_core : (c + 1) * rays_per_core]
            nch = np.maximum(1, -(-cc // cap)).astype(np.int32)
            got = _pack_core(nch, spr, bounds)
            if got is None:
                ok = False
                break
            packs.append(got)
        if ok:
            break
        spr += 1
    F = spr * L

    key = (L, spr, first, spc, shift_f, interval_f)
    if key not in _nc_cache:
        _nc_cache[key] = _build_nc(L, spr, first, spc, shift_f, interval_f)
    nc = _nc_cache[key]

    # per-ray placement arrays (global)
    g_row = np.empty(rays_padded, dtype=np.int64)
    g_slot0 = np.empty(rays_padded, dtype=np.int64)
    for c in range(N_CORES):
        row, slot0 = packs[c]
        g_row[c * rays_per_core : (c + 1) * rays_per_core] = row + c * P
        g_slot0[c * rays_per_core : (c + 1) * rays_per_core] = slot0

    # per-sample positions in the concatenated [8*128, F] padded array
    starts = np.zeros(rays_padded + 1, dtype=np.int64)
    np.cumsum(counts, out=starts[1:])
    off = np.arange(M, dtype=np.int64) - starts[ray_id]
    pos = (
        g_row[ray_id] * F
        + (g_slot0[ray_id] + off // cap) * L
        + off % cap
    )

    padded = np.full(N_CORES * P * F, PAD_VAL, dtype=np.float32)
    padded[pos] = density
    padded = padded.reshape(N_CORES, P, F)

    # continuation mask: 1.0 at every non-first chunk slot of a ray
    contmask = np.zeros((N_CORES * P, spr), dtype=np.float32)
    multi = np.nonzero(counts > cap)[0]
    for r in multi:
        for j in range(1, int(-(-counts[r] // cap))):
            contmask[g_row[r], g_slot0[r] + j] = 1.0
    contmask = contmask.reshape(N_CORES, P, spr)

    in_maps = [
        {"density": padded[c], "contmask": contmask[c]} for c in range(N_CORES)
    ]
    res = run_bass_kernel_spmd(
        nc,
        in_maps,
        core_ids=list(range(N_CORES)),
        trace=bool(os.environ.get("BASS_TRACE")),
    )
    if _bench is not None:
        _bench.append(res)

    w_all = np.stack([res.results[c]["weights"] for c in range(N_CORES)])
    weights = np.ascontiguousarray(w_all.reshape(-1)[pos])
    # alphainv: running product at the ray's LAST chunk's slot end
    ai_all = np.stack([res.results[c]["alphainv"] for c in range(N_CORES)])
    nch_all = np.maximum(1, -(-counts // cap)).astype(np.int64)
    ai_pos = g_row * spr + g_slot0 + nch_all - 1
    alphainv = np.ascontiguousarray(ai_all.reshape(-1)[ai_pos][:n_rays])
    return weights, alphainv
